# revision 1
# baseline (speedup 1.0000x reference)
"""DialogueGCN windowed-attention relational GCN on 8 Trainium2 NeuronCores.

Sharding: utterance axis N=16384 split into 8 shards of 2048 rows; each core
gets its shard plus a 128-row halo on each side (zero-padded at the global
edges). The small DxD weights are replicated. No collectives needed.

Per-core algorithm (banded ops as dense matmuls over a 2x128-row jj-window
per 128-row output block; supports live on a 64-row-shifted chunk grid so
each block's 255-row band is exactly two aligned support chunks):
  xT    = host-transposed x_halo, f32r-rounded on device
  qT    = (x @ W_att)^T                     (via W_att chunks vs xT)
  S[r]  = x_halo @ Wr_combined, r in a,b,c  (3 supports via mask linearity;
                                             S_a carries a ones-column that
                                             yields the softmax denominator)
  per block b:
    R[nl, jj] = sum_d qT[d, n] xT[d, j]     (attention logits, [128, 256])
    E     = exp(R - rowmax(R))              (ScalarE; band masking deferred
                                             to the transposed strips)
    Et    = E^T                             (PE transpose pair -> one PSUM)
    c1/c2 = Et * band / suc strip masks, c3 = Et * same-speaker (DVE stt)
    h|esum = 6 accumulating matmuls strips^T-contract S[..]
    log_softmax over d finalized per block-pair straight out of PSUM, with
    rinv folded into activation scale operands; ScalarE runs one pre-seeded
    activation table (set 6: exp+ln+copy+identity) for the whole kernel.

GEMM-phase chunks are interleaved with output blocks in emission order so
every scheduling window carries PE, DVE, ACT, and POOL work.
"""

import os
import numpy as np

N_TOT, D, W, SPK = 16384, 256, 64, 8
NCORES = 8
NC_ROWS = N_TOT // NCORES          # 2048 rows per core
HALO = 128
NH = NC_ROWS + 2 * HALO            # 2304 rows with halo
NBLK = NC_ROWS // 128              # 16 output blocks per core
NCH = NH // 128                    # 18 halo chunks (aligned grid)
NSH = NCH - 1                      # 17 chunks on the 64-shifted grid
NEG_BIG = -1.0e30

_cache = {}


def _build_bass():
    import concourse.tile as tile
    from concourse import bacc, mybir

    f32 = mybir.dt.float32
    f32r = mybir.dt.float32r
    bf16 = mybir.dt.bfloat16
    AX = mybir.AxisListType.X
    OP = mybir.AluOpType
    AF = mybir.ActivationFunctionType

    nc = bacc.Bacc("TRN2", target_bir_lowering=False, debug=False,
                   num_devices=NCORES)

    xt_d = nc.dram_tensor("xt", [2, 128, NH], f32, kind="ExternalInput").ap()
    spk_d = nc.dram_tensor("spk", [NH], f32, kind="ExternalInput").ap()
    wq_d = nc.dram_tensor("wq", [D, D], f32, kind="ExternalInput").ap()
    wa_d = nc.dram_tensor("wa", [D, D], f32, kind="ExternalInput").ap()
    wb_d = nc.dram_tensor("wb", [D, D], f32, kind="ExternalInput").ap()
    wc_d = nc.dram_tensor("wc", [D, D], f32, kind="ExternalInput").ap()
    band_d = nc.dram_tensor("band", [128, 256], f32, kind="ExternalInput").ap()
    sucm_d = nc.dram_tensor("sucm", [128, 256], f32, kind="ExternalInput").ap()
    ident_d = nc.dram_tensor("ident", [128, 128], f32, kind="ExternalInput").ap()
    out_d = nc.dram_tensor("out", [NC_ROWS, D], f32, kind="ExternalOutput").ap()

    with tile.TileContext(nc) as tc:
        from contextlib import ExitStack
        with ExitStack() as ctx:
            const = ctx.enter_context(tc.tile_pool(name="const", bufs=1))
            persist = ctx.enter_context(tc.tile_pool(name="persist", bufs=1))
            work = ctx.enter_context(tc.tile_pool(name="work", bufs=int(os.environ.get("KB_WORK", "5"))))
            psum = ctx.enter_context(tc.tile_pool(name="psum", bufs=2, space="PSUM"))

            # Pre-seed the activation-function table with the one set that
            # covers every func used here (exp, ln, copy, identity), so the
            # table-load pass never has to reload mid-kernel.
            nc.scalar.add_instruction(mybir.InstLoadActFuncSet(
                name=nc.get_next_instruction_name(), ins=[], outs=[],
                act_func_set_id=6))

            # ---- DMA order tuned for the first dependent matmuls: wq and
            # the first x slices gate qT(0); support weights follow ----
            w_r = {}

            def load_w(name, wd):
                stage = work.tile([128, 2, D], f32, tag="wstage", name="stage")
                nc.sync.dma_start(stage, wd.rearrange("(k p) d -> p k d", p=128))
                wr = const.tile([128, 2, D], f32r, name=f"{name}_r")
                nc.vector.tensor_copy(wr, stage)
                w_r[name] = wr

            load_w("wq", wq_d)
            ident_sb = const.tile([128, 128], f32)
            nc.sync.dma_start(ident_sb, ident_d)
            xts = persist.tile([128, 2, NH], f32)
            xt_v = xt_d.rearrange("k p n -> p k n")
            nsplit = int(os.environ.get("KB_XSPLIT", "4"))
            for g in range(nsplit):
                nc.sync.dma_start(xts[:, :, g * 256:(g + 1) * 256],
                                  xt_v[:, :, g * 256:(g + 1) * 256])
            load_w("wa", wa_d)
            load_w("wb", wb_d)
            load_w("wc", wc_d)
            for g in range(nsplit, NCH // 2):
                nc.sync.dma_start(xts[:, :, g * 256:(g + 1) * 256],
                                  xt_v[:, :, g * 256:(g + 1) * 256])

            band_f = work.tile([128, 256], f32, tag="wstage2")
            nc.sync.dma_start(band_f, band_d)
            band_sb = const.tile([128, 256], bf16)
            nc.vector.tensor_copy(band_sb, band_f)
            sucm_f = work.tile([128, 256], f32, tag="wstage2")
            nc.sync.dma_start(sucm_f, sucm_d)
            sucm_sb = const.tile([128, 256], bf16)
            nc.vector.tensor_copy(sucm_sb, sucm_f)
            ident_r = const.tile([128, 128], f32r)
            nc.vector.tensor_copy(ident_r, ident_sb)
            ident_b = const.tile([128, 128], bf16)
            nc.vector.tensor_copy(ident_b, ident_sb)

            # speakers: shifted column layout [128, NSH] and broadcast rows
            spk_col = persist.tile([128, NSH], f32)
            nc.sync.dma_start(
                spk_col, spk_d[64:64 + NSH * 128].rearrange("(c p) -> p c", p=128))
            spk_row = persist.tile([1, NC_ROWS], f32)
            nc.sync.dma_start(
                spk_row, spk_d.rearrange("(a b) -> a b", a=1)[:, HALO:HALO + NC_ROWS])
            spk_rowb = persist.tile([1, NC_ROWS], bf16)
            nc.vector.tensor_copy(spk_rowb, spk_row)
            spk_bc = persist.tile([128, NC_ROWS], bf16)
            nc.gpsimd.partition_broadcast(spk_bc, spk_rowb)

            # ---- xT: f32r rounding copies of the host-transposed x ----
            xT = persist.tile([128, 2, NH], f32r)
            for c2 in range(NCH // 2):
                csl = slice(c2 * 256, (c2 + 1) * 256)
                xmode = os.environ.get("KB_XT", "parity")
                if xmode == "pool" or (xmode == "mix" and c2 >= 3):
                    xeng = nc.gpsimd.tensor_copy
                elif xmode in ("act", "mix") or (xmode == "parity" and c2 % 2 == 0):
                    xeng = nc.scalar.copy
                else:
                    xeng = nc.vector.tensor_copy
                xeng(xT[:, :, csl], xts[:, :, csl])

            qT = persist.tile([128, 2, NC_ROWS], f32r)
            S = persist.tile([128, 3, NSH, 264], bf16)
            nc.gpsimd.memset(S[:, 0, :, 256:257], 1.0)

            # ---- staging for the log_softmax tail ----
            s2_all = persist.tile([128, NBLK], f32)
            rinv_all = persist.tile([128, NBLK], f32)

            # ---- qT: one 512-column group ----
            def emit_qT(g):
                nsl = slice(HALO + g * 512, HALO + (g + 1) * 512)
                for dh in (0, 1):
                    psq = psum.tile([128, 512], f32, tag="ph", name="psq", bufs=int(os.environ.get("KB_PH", "2")))
                    for k in (0, 1):
                        nc.tensor.matmul(
                            psq, w_r["wq"][:, k, dh * 128:(dh + 1) * 128],
                            xT[:, k, nsl], start=(k == 0), stop=(k == 1))
                    qmode = os.environ.get("KB_QT", "act")
                    qeng = nc.scalar.copy if (
                        qmode == "act" or (qmode == "parity" and dh == 0)
                    ) else nc.vector.tensor_copy
                    qeng(qT[:, dh, g * 512:(g + 1) * 512], psq)

            # ---- one support chunk on the 64-shifted grid ----
            def emit_S(c):
                csl = slice(64 + c * 128, 64 + (c + 1) * 128)
                pab = psum.tile([128, 512], f32, tag="ph", name="pab", bufs=int(os.environ.get("KB_PH", "2")))
                for i, name in enumerate(("wa", "wb")):
                    for k in (0, 1):
                        nc.tensor.matmul(
                            pab[:, i * 256:(i + 1) * 256], xT[:, k, csl],
                            w_r[name][:, k, :], start=(k == 0), stop=(k == 1))
                pab_v = pab.rearrange("p (i d) -> p i d", i=2)
                smode = os.environ.get("KB_SP", "parity")
                if smode == "act" or (smode == "parity" and c % 2 == 0):
                    nc.scalar.copy(S[:, 0:2, c, 0:D], pab_v)
                else:
                    nc.vector.tensor_copy(S[:, 0:2, c, 0:D], pab_v)
                pwc = psum.tile([128, D], f32, tag="ph", name="pwc", bufs=int(os.environ.get("KB_PH", "2")))
                for k in (0, 1):
                    nc.tensor.matmul(pwc, xT[:, k, csl], w_r["wc"][:, k, :],
                                     start=(k == 0), stop=(k == 1))
                wmode = os.environ.get("KB_WC", "dve")
                if wmode == "act" or (wmode == "parity" and c % 2 == 0):
                    nc.scalar.copy(S[:, 2, c, 0:D], pwc)
                else:
                    nc.vector.tensor_copy(S[:, 2, c, 0:D], pwc)

            # ---- one 128-row output block ----
            def emit_block(b):
                nsl = slice(b * 128, (b + 1) * 128)
                # attention logits R [128, 256]: jj-window = halo cols
                # [b*128+64, b*128+320)
                psr = psum.tile([128, 256], f32, tag="psr", name="psr",
                                bufs=int(os.environ.get("KB_PSR", "2")))
                for k in (0, 1):
                    nc.tensor.matmul(psr, qT[:, k, nsl],
                                     xT[:, k, b * 128 + 64: b * 128 + 320],
                                     start=(k == 0), stop=(k == 1))
                # e = exp(R - rowmax(R)) over the full jj window; the band
                # mask is applied to the transposed strips instead, and the
                # softmax denominator comes out of the aggregation matmul via
                # a ones-column appended to S_a (softmax shift invariance).
                negmax = work.tile([128, 1], f32, tag="negmax")
                nc.vector.reduce_max(negmax, psr, axis=AX, negate=True)
                ee = work.tile([128, 256], bf16, tag="ee")
                nc.scalar.activation(ee, psr, AF.Exp, bias=negmax)

                # transposed strip pair Et [128, 256] (cols 0:128 = chunk A)
                pte = psum.tile([128, 256], bf16, tag="pte", name="pte",
                                bufs=int(os.environ.get("KB_PTE", "2")))
                for c in (0, 1):
                    nc.tensor.transpose(pte[:, c * 128:(c + 1) * 128],
                                        ee[:, c * 128:(c + 1) * 128], ident_b)
                et = work.tile([128, 256], bf16, tag="et")
                etmode = os.environ.get("KB_ET", "dve")
                if etmode == "act" or (etmode == "parity" and b % 2 == 0):
                    nc.scalar.copy(et, pte)
                else:
                    nc.vector.tensor_copy(et, pte)

                # banded strip (POOL), direction strip (POOL), speaker (DVE)
                c1 = work.tile([128, 256], bf16, tag="c1")
                nc.gpsimd.tensor_tensor(c1, et, band_sb, op=OP.mult)
                c2 = work.tile([128, 256], bf16, tag="c2")
                c2eng = nc.gpsimd if os.environ.get("KB_C2", "dve") == "pool" else nc.vector
                c2eng.tensor_tensor(c2, et, sucm_sb, op=OP.mult)
                c3 = work.tile([128, 256], bf16, tag="c3")
                for c in (0, 1):
                    nc.vector.scalar_tensor_tensor(
                        c3[:, c * 128:(c + 1) * 128],
                        in0=spk_bc[:, nsl], scalar=spk_col[:, b + c:b + c + 1],
                        in1=c1[:, c * 128:(c + 1) * 128],
                        op0=OP.is_equal, op1=OP.mult)

                # aggregation (+ softmax denominator in column 256)
                psh = psum.tile([128, 257], f32, tag="psh", name="psh",
                                bufs=int(os.environ.get("KB_PSH", "2")))
                mms = [(c1, 0, 0), (c1, 1, 0), (c3, 0, 2), (c3, 1, 2),
                       (c2, 0, 1), (c2, 1, 1)]
                for i, (strip, c, r) in enumerate(mms):
                    wid = 257 if r == 0 else D
                    nc.tensor.matmul(psh[:, 0:wid],
                                     strip[:, c * 128:(c + 1) * 128],
                                     S[:, r, b + c, 0:wid],
                                     start=(i == 0), stop=(i == len(mms) - 1),
                                     skip_group_check=True)

                psh_hist[b] = psh
                rinv = rinv_all[:, b:b + 1]
                nc.vector.reciprocal(rinv, psh[:, 256:257])
                e2 = work.tile([128, D], f32, tag="e2")
                nc.scalar.activation(e2, psh[:, 0:D], AF.Exp,
                                     scale=rinv,
                                     accum_out=s2_all[:, b:b + 1])

                # finalize a pair of blocks straight out of PSUM (Ln shares
                # the Exp activation table set -> no reloads)
                if b % 2 == 1:
                    g = b // 2
                    gs = slice(g * 2, g * 2 + 2)
                    ln2 = work.tile([128, 2], f32, tag="ln2")
                    nc.scalar.activation(ln2, s2_all[:, gs], AF.Ln)
                    bias2 = work.tile([128, 2], f32, tag="bias2")
                    nc.vector.tensor_scalar_mul(bias2, ln2, -1.0)
                    ob2 = work.tile([128, 2, D], f32, tag="ob2")
                    for i in range(2):
                        bb = 2 * g + i
                        if os.environ.get("KB_OB", "act") == "act":
                            nc.scalar.activation(
                                ob2[:, i, :], psh_hist[bb][:, 0:D], AF.Identity,
                                bias=bias2[:, i:i + 1],
                                scale=rinv_all[:, bb:bb + 1])
                        else:
                            nc.vector.tensor_scalar(
                                ob2[:, i, :], psh_hist[bb][:, 0:D],
                                scalar1=rinv_all[:, bb:bb + 1],
                                scalar2=bias2[:, i:i + 1],
                                op0=OP.mult, op1=OP.add)
                    nc.sync.dma_start(
                        out_d.rearrange("(c p) d -> p c d", p=128)[:, gs, :], ob2)

            psh_hist = {}
            # ---- interleaved driver: mix GEMM phases with block groups so
            # every scheduling window has PE, DVE, ACT, and POOL work ----
            if os.environ.get("KB_STREAMS", "1") == "2":
                # two independent block streams (lower/upper shard half) give
                # the scheduler unrelated work to fill dependency bubbles
                emitted = set()

                def emit_S_range(lo, hi):
                    for c in range(lo, hi):
                        if c not in emitted:
                            emit_S(c)
                            emitted.add(c)

                for half in range(2):
                    b0 = half * 4           # lower-stream group
                    b1 = half * 4 + 8       # upper-stream group
                    emit_qT(half)
                    emit_qT(half + 2)
                    emit_S_range(b0, b0 + 5)
                    emit_S_range(b1, min(b1 + 5, NSH))
                    for i in range(4):
                        emit_block(b0 + i)
                        emit_block(b1 + i)
            else:
                s_next = 0
                look = int(os.environ.get("KB_LOOK", "1"))
                for g in range(NBLK // 4):
                    emit_qT(g)
                    hi = min(4 * (g + 1) + look, NSH)
                    while s_next < hi:
                        emit_S(s_next)
                        s_next += 1
                    for i in range(4):
                        emit_block(4 * g + i)

    nc.compile()
    return nc


def _host_constants():
    # strip-space masks: chunk A has j = n0 - 64 + p, chunk B j = n0 + 64 + p,
    # column f = local output row. In-band means w = j - n + 64 in [0, 128).
    p = np.arange(128)[:, None]
    f = np.arange(128)[None, :]
    band = np.concatenate([(p >= f), (p < f)], axis=1).astype(np.float32)
    suc = np.concatenate([(f <= p) & (p < f + 64), (p < f - 64)],
                         axis=1).astype(np.float32)
    ident = np.eye(128, dtype=np.float32)
    return band, suc, ident


def _prep_in_maps(np_inputs):
    x = np.asarray(np_inputs["x"], dtype=np.float32)
    spk = np.asarray(np_inputs["speaker_ids"]).astype(np.float32)
    W_att = np.asarray(np_inputs["W_att"], dtype=np.float32)
    W_pred = np.asarray(np_inputs["W_pred"], dtype=np.float32)
    W_suc = np.asarray(np_inputs["W_suc"], dtype=np.float32)
    W_same = np.asarray(np_inputs["W_same"], dtype=np.float32)
    W_diff = np.asarray(np_inputs["W_diff"], dtype=np.float32)

    band, sucm, ident = _host_constants()
    wa = W_pred + W_diff
    wb = W_suc - W_pred
    wc = W_same - W_diff

    xp = np.zeros((N_TOT + 2 * HALO, D), dtype=np.float32)
    xp[HALO:HALO + N_TOT] = x
    spkp = np.full((N_TOT + 2 * HALO,), -1.0, dtype=np.float32)
    spkp[HALO:HALO + N_TOT] = spk

    in_maps = []
    for k in range(NCORES):
        r0 = k * NC_ROWS
        in_maps.append({
            "xt": np.ascontiguousarray(
                xp[r0:r0 + NH].T.reshape(2, 128, NH)),
            "spk": np.ascontiguousarray(spkp[r0:r0 + NH]),
            "wq": W_att, "wa": wa, "wb": wb, "wc": wc,
            "band": band, "sucm": sucm, "ident": ident,
        })
    return in_maps


def kernel(x, speaker_ids, W_att, W_pred, W_suc, W_same, W_diff):
    from concourse import bass_utils

    if "nc" not in _cache:
        _cache["nc"] = _build_bass()
    nc = _cache["nc"]

    in_maps = _prep_in_maps({
        "x": x, "speaker_ids": speaker_ids, "W_att": W_att, "W_pred": W_pred,
        "W_suc": W_suc, "W_same": W_same, "W_diff": W_diff})

    res = bass_utils.run_bass_kernel_spmd(nc, in_maps, core_ids=list(range(NCORES)))
    _cache["last_result"] = res
    return np.concatenate([res.results[k]["out"] for k in range(NCORES)], axis=0)



# revision 5
# speedup vs baseline: 1.0594x; 1.0594x over previous
"""DialogueGCN windowed-attention relational GCN on 8 Trainium2 NeuronCores.

Sharding: utterance axis N=16384 split into 8 shards of 2048 rows; each core
gets its shard plus a 128-row halo on each side (zero-padded at the global
edges). The small DxD weights are replicated. No collectives needed.

v2: chunk-centric, transpose-free pipeline. All banded ops are dense matmuls
on a 64-row-shifted chunk grid (17 chunks of 128 halo rows per core), with
attention logits computed DIRECTLY TRANSPOSED into strip space [jj, n]:

  qT        = (x @ W_att)^T            (f32r, from host-transposed x)
  S[r]      = x @ Wr_combined          (3 supports via mask linearity, bf16
                                        storage; S_a carries a ones column
                                        that yields the softmax denominator)
  per chunk c (jj rows), as pairs sharing one PSUM bank:
    T[jj,n] = sum_d xT[d,jj] qT[d,n]   (256 n-cols spanning 2 blocks)
    e       = exp(T - C)               (ScalarE, constant shift: softmax
                                        shift-invariance makes a per-row max
                                        unnecessary in f32/bf16 range)
    c1      = e * band01               (band mask, DVE; c2 = c1 * suc01,
                                        c3 = same-speaker stt on c1)
  per block b: 6 accumulating bf16 matmuls strips^T-contract S -> h | denom,
  then log_softmax finalized per block pair straight out of PSUM with rinv
  folded into activation scale operands.

No PE transposes, no row-max reductions, no f32r rounding copies (f32 DMAs
land directly in f32r tiles).
"""

import os
import numpy as np

N_TOT, D, W, SPK = 16384, 256, 64, 8
NCORES = 8
NC_ROWS = N_TOT // NCORES          # 2048 rows per core
HALO = 128
NH = NC_ROWS + 2 * HALO            # 2304 rows with halo
NBLK = NC_ROWS // 128              # 16 output blocks per core
NCH = NH // 128                    # 18 aligned chunks
NSH = NCH - 1                      # 17 chunks on the 64-shifted grid
NPAIR = (NSH + 1) // 2             # 9 strip pairs (last one is a single)
C_SHIFT = 30.0

_cache = {}


def _build_bass():
    import concourse.tile as tile
    from concourse import bacc, mybir

    f32 = mybir.dt.float32
    f32r = mybir.dt.float32r
    bf16 = mybir.dt.bfloat16
    AX = mybir.AxisListType.X
    OP = mybir.AluOpType
    AF = mybir.ActivationFunctionType

    nc = bacc.Bacc("TRN2", target_bir_lowering=False, debug=False,
                   num_devices=NCORES)

    xt_d = nc.dram_tensor("xt", [2, 128, NH], f32r, kind="ExternalInput").ap()
    spk_d = nc.dram_tensor("spk", [NH], f32, kind="ExternalInput").ap()
    wq_d = nc.dram_tensor("wq", [128, 2, D], f32r, kind="ExternalInput").ap()
    wabc_d = nc.dram_tensor("wabc", [128, 2, 3, D], f32r, kind="ExternalInput").ap()
    band_d = nc.dram_tensor("band01", [128, 512], bf16, kind="ExternalInput").ap()
    suc_d = nc.dram_tensor("suc01", [128, 512], bf16, kind="ExternalInput").ap()
    out_d = nc.dram_tensor("out", [NC_ROWS, D], f32, kind="ExternalOutput").ap()

    qpad = 128                     # zero columns either side of qT

    with tile.TileContext(nc) as tc:
        from contextlib import ExitStack
        with ExitStack() as ctx:
            const = ctx.enter_context(tc.tile_pool(name="const", bufs=1))
            persist = ctx.enter_context(tc.tile_pool(name="persist", bufs=1))
            work = ctx.enter_context(tc.tile_pool(
                name="work", bufs=int(os.environ.get("KB_WORK", "4"))))
            psum = ctx.enter_context(tc.tile_pool(name="psum", bufs=2, space="PSUM"))

            # Pre-seed the activation table set covering Exp + Ln so the
            # table-load pass never reloads mid-kernel.
            nc.scalar.add_instruction(mybir.InstLoadActFuncSet(
                name=nc.get_next_instruction_name(), ins=[], outs=[],
                act_func_set_id=6))

            # ---- input DMAs, ordered so qT(0) can start ASAP ----
            wq = const.tile([128, 2, D], f32r)
            nc.sync.dma_start(wq, wq_d)
            xT = persist.tile([128, 2, NH], f32r)
            xt_v = xt_d.rearrange("k p n -> p k n")
            nxs = int(os.environ.get("KB_XSPLIT", "4"))
            xsz = NH // nxs
            for g in range(nxs):
                nc.sync.dma_start(xT[:, :, g * xsz:(g + 1) * xsz],
                                  xt_v[:, :, g * xsz:(g + 1) * xsz])
            wabc = const.tile([128, 2, 3, D], f32r)
            nc.sync.dma_start(wabc, wabc_d)
            band01 = const.tile([128, 512], bf16)
            nc.sync.dma_start(band01, band_d)
            suc01 = const.tile([128, 512], bf16)
            nc.sync.dma_start(suc01, suc_d)

            # speakers: per-chunk column layout and broadcast rows
            spk_col = persist.tile([128, NSH], f32)
            nc.sync.dma_start(
                spk_col, spk_d[64:64 + NSH * 128].rearrange("(c p) -> p c", p=128))
            spk_row = persist.tile([1, NC_ROWS], f32)
            nc.sync.dma_start(
                spk_row, spk_d.rearrange("(a b) -> a b", a=1)[:, HALO:HALO + NC_ROWS])
            spk_rowb = persist.tile([1, NC_ROWS], bf16)
            nc.vector.tensor_copy(spk_rowb, spk_row)
            spk_bc = persist.tile([128, NC_ROWS], bf16)
            nc.gpsimd.partition_broadcast(spk_bc, spk_rowb)

            # qT padded with zero columns so every strip matmul is 256 wide
            qT = persist.tile([128, 2, qpad + NC_ROWS + qpad], f32r)
            nc.gpsimd.memset(qT[:, :, 0:qpad], 0.0)
            nc.gpsimd.memset(qT[:, :, qpad + NC_ROWS:], 0.0)

            # supports: [p, r, chunk, 264] bf16; S_a ones column at 256
            S = persist.tile([128, 3, NSH, 264], bf16)
            nc.gpsimd.memset(S[:, 0, :, 256:257], 1.0)

            # constant exp shift (softmax shift-invariance, no row max)
            cbias = const.tile([128, 1], f32)
            nc.gpsimd.memset(cbias, -C_SHIFT)

            # log-softmax tail staging
            s2_all = persist.tile([128, NBLK], f32)
            rinv_all = persist.tile([128, NBLK], f32)

            # ---- qT: one 512-column group (psq shares PSUM tag with strips) ----
            def emit_qT(g):
                nsl = slice(HALO + g * 512, HALO + (g + 1) * 512)
                for dh in (0, 1):
                    psq = psum.tile([128, 512], f32, tag="big", name="psq",
                                    bufs=int(os.environ.get("KB_PBIG", "2")))
                    for k in (0, 1):
                        nc.tensor.matmul(
                            psq, wq[:, k, dh * 128:(dh + 1) * 128],
                            xT[:, k, nsl], start=(k == 0), stop=(k == 1))
                    qeng = nc.scalar.copy if (g + dh) % 2 else nc.vector.tensor_copy
                    qeng(qT[:, dh, qpad + g * 512:qpad + (g + 1) * 512], psq)

            # ---- one support chunk on the 64-shifted grid ----
            sc_state = {}

            def emit_S(c):
                csl = slice(64 + c * 128, 64 + (c + 1) * 128)
                pab = psum.tile([128, 512], f32, tag="sab", name="pab",
                                bufs=int(os.environ.get("KB_PSAB", "2")))
                for k in (0, 1):
                    for i in range(2):
                        nc.tensor.matmul(
                            pab[:, i * 256:(i + 1) * 256], xT[:, k, csl],
                            wabc[:, k, i, :], start=(k == 0), stop=(k == 1),
                            skip_group_check=True)
                smode = os.environ.get("KB_SP", "parity")
                if smode == "act" or (smode == "parity" and c % 2 == 0):
                    seng = nc.scalar.copy
                elif smode == "pool" or (smode == "parity2" and c % 2 == 0):
                    seng = nc.gpsimd.tensor_copy
                else:
                    seng = nc.vector.tensor_copy
                seng(S[:, 0:2, c, 0:D], pab.rearrange("p (i d) -> p i d", i=2))

                # S_c: two chunks share one PSUM bank
                half = c % 2
                if half == 0:
                    sc_state["t"] = psum.tile([128, 512], f32, tag="sc", name="psc",
                                              bufs=int(os.environ.get("KB_PSC", "2")))
                psc = sc_state["t"]
                for k in (0, 1):
                    nc.tensor.matmul(psc[:, half * 256:(half + 1) * 256],
                                     xT[:, k, csl], wabc[:, k, 2, :],
                                     start=(k == 0), stop=(k == 1),
                                     skip_group_check=True)
                if half == 1 or c == NSH - 1:
                    c0 = c - half
                    n = half + 1
                    wmode = os.environ.get("KB_WC", "pool")
                    weng = (nc.gpsimd.tensor_copy if wmode == "pool" else
                            nc.vector.tensor_copy if wmode == "dve" else
                            nc.scalar.copy)
                    weng(S[:, 2, c0:c0 + n, 0:D],
                         psc[:, 0:n * 256].rearrange("p (i d) -> p i d", i=n))

            # ---- strip pair P: chunks 2P, 2P+1 -> c1/c2/c3 [128, 512] ----
            strips = {}

            def emit_strips(P):
                cs = [c for c in (2 * P, 2 * P + 1) if c < NSH]
                n = len(cs)
                pt = psum.tile([128, 512], f32, tag="big", name="pt",
                               bufs=int(os.environ.get("KB_PBIG", "2")))
                for i, c in enumerate(cs):
                    jsl = slice(64 + c * 128, 64 + (c + 1) * 128)
                    qsl = slice(qpad + (c - 1) * 128, qpad + (c + 1) * 128)
                    for k in (0, 1):
                        nc.tensor.matmul(pt[:, i * 256:(i + 1) * 256],
                                         xT[:, k, jsl], qT[:, k, qsl],
                                         start=(k == 0), stop=(k == 1),
                                         skip_group_check=True)
                w = n * 256
                c1 = work.tile([128, 512], bf16, tag="c1", name="c1")
                nc.scalar.activation(c1[:, 0:w], pt[:, 0:w], AF.Exp,
                                     bias=cbias)
                nc.vector.tensor_tensor(c1[:, 0:w], c1[:, 0:w], band01[:, 0:w],
                                        op=OP.mult)
                c2 = work.tile([128, 512], bf16, tag="c2", name="c2")
                c2eng = nc.gpsimd if os.environ.get("KB_C2", "dve") == "pool" else nc.vector
                c2eng.tensor_tensor(c2[:, 0:w], c1[:, 0:w], suc01[:, 0:w],
                                    op=OP.mult)
                c3 = work.tile([128, 512], bf16, tag="c3", name="c3")
                for i, c in enumerate(cs):
                    nlo = (c - 1) * 128
                    s0 = max(0, -nlo)
                    s1 = min(256, NC_ROWS - nlo)
                    nc.vector.scalar_tensor_tensor(
                        c3[:, i * 256 + s0:i * 256 + s1],
                        in0=spk_bc[:, nlo + s0:nlo + s1],
                        scalar=spk_col[:, c:c + 1],
                        in1=c1[:, i * 256 + s0:i * 256 + s1],
                        op0=OP.is_equal, op1=OP.mult)
                strips[P] = (c1, c2, c3)

            # ---- one 128-row output block ----
            psh_hist = {}

            def strip_slice(tile_trip, c, b):
                # columns of chunk c's strip belonging to block b
                i = c % 2
                f0 = (b - (c - 1)) * 128           # 128 if c==b, 0 if c==b+1
                return tuple(t[:, i * 256 + f0:i * 256 + f0 + 128]
                             for t in tile_trip)

            def emit_block(b):
                psh = psum.tile([128, 257], f32, tag="psh", name="psh",
                                bufs=int(os.environ.get("KB_PSH", "2")))
                mms = []
                for c in (b, b + 1):
                    c1s, c2s, c3s = strip_slice(strips[c // 2], c, b)
                    mms.append((c1s, 0, c))
                    mms.append((c3s, 2, c))
                    mms.append((c2s, 1, c))
                for i, (st, r, c) in enumerate(mms):
                    wid = 257 if r == 0 else D
                    nc.tensor.matmul(psh[:, 0:wid], st, S[:, r, c, 0:wid],
                                     start=(i == 0), stop=(i == len(mms) - 1),
                                     skip_group_check=True)
                psh_hist[b] = psh
                rinv = rinv_all[:, b:b + 1]
                nc.vector.reciprocal(rinv, psh[:, 256:257])
                e2 = work.tile([128, D], bf16, tag="e2")
                nc.scalar.activation(e2, psh[:, 0:D], AF.Exp, scale=rinv,
                                     accum_out=s2_all[:, b:b + 1])

                if b % 2 == 1:
                    g = b // 2
                    gs = slice(g * 2, g * 2 + 2)
                    ln2 = work.tile([128, 2], f32, tag="ln2")
                    nc.scalar.activation(ln2, s2_all[:, gs], AF.Ln)
                    bias2 = work.tile([128, 2], f32, tag="bias2")
                    nc.vector.tensor_scalar_mul(bias2, ln2, -1.0)
                    ob2 = work.tile([128, 2, D], f32, tag="ob2")
                    obm = os.environ.get("KB_OB", "dve")
                    for i in range(2):
                        bb = 2 * g + i
                        if obm == "act":
                            nc.scalar.activation(
                                ob2[:, i, :], psh_hist[bb][:, 0:D], AF.Identity,
                                bias=bias2[:, i:i + 1],
                                scale=rinv_all[:, bb:bb + 1])
                        else:
                            eng = nc.gpsimd if obm == "pool" else nc.vector
                            eng.tensor_scalar(
                                ob2[:, i, :], psh_hist[bb][:, 0:D],
                                scalar1=rinv_all[:, bb:bb + 1],
                                scalar2=bias2[:, i:i + 1],
                                op0=OP.mult, op1=OP.add)
                        del psh_hist[bb]
                    nc.sync.dma_start(
                        out_d.rearrange("(c p) d -> p c d", p=128)[:, gs, :], ob2)

            # ---- driver: pipelined emission ----
            for P in range(NPAIR):
                if P < 4:
                    emit_qT(P)
                emit_S(2 * P)
                if 2 * P + 1 < NSH:
                    emit_S(2 * P + 1)
                emit_strips(P)
                for b in (2 * P - 2, 2 * P - 1):
                    if 0 <= b < NBLK:
                        emit_block(b)

    nc.compile()
    return nc


def _host_constants():
    # strip-space masks for a chunk pair [128, 2, 256]: within a chunk,
    # column f covers n = 128*(c-1)+f; row p covers j = 128*c+p-64 (shard
    # coords). In-band: j-n in [-64, 64) <=> p < f <= p+128; successor
    # (j < n): f - p in (64, 128].
    p = np.arange(128)[:, None]
    f = np.arange(256)[None, :]
    band = ((p < f) & (f <= p + 128)).astype(np.float32)
    suc = ((f - p > 64) & (f - p <= 128)).astype(np.float32)
    band2 = np.concatenate([band, band], axis=1)
    suc2 = np.concatenate([suc, suc], axis=1)
    return band2, suc2


def _prep_in_maps(np_inputs):
    import ml_dtypes
    bf = ml_dtypes.bfloat16
    x = np.asarray(np_inputs["x"], dtype=np.float32)
    spk = np.asarray(np_inputs["speaker_ids"]).astype(np.float32)
    W_att = np.asarray(np_inputs["W_att"], dtype=np.float32)
    W_pred = np.asarray(np_inputs["W_pred"], dtype=np.float32)
    W_suc = np.asarray(np_inputs["W_suc"], dtype=np.float32)
    W_same = np.asarray(np_inputs["W_same"], dtype=np.float32)
    W_diff = np.asarray(np_inputs["W_diff"], dtype=np.float32)

    band2, suc2 = _host_constants()
    wa = W_pred + W_diff
    wb = W_suc - W_pred
    wc = W_same - W_diff
    # k-tile layouts [p, k, ...]
    wq_kt = np.ascontiguousarray(W_att.reshape(2, 128, D).transpose(1, 0, 2))
    wabc = np.stack([wa, wb, wc], axis=1)              # [256, 3, D]
    wabc_kt = np.ascontiguousarray(
        wabc.reshape(2, 128, 3, D).transpose(1, 0, 2, 3))

    xp = np.zeros((N_TOT + 2 * HALO, D), dtype=np.float32)
    xp[HALO:HALO + N_TOT] = x
    spkp = np.full((N_TOT + 2 * HALO,), -1.0, dtype=np.float32)
    spkp[HALO:HALO + N_TOT] = spk

    in_maps = []
    for k in range(NCORES):
        r0 = k * NC_ROWS
        in_maps.append({
            "xt": np.ascontiguousarray(
                xp[r0:r0 + NH].T.reshape(2, 128, NH)),
            "spk": np.ascontiguousarray(spkp[r0:r0 + NH]),
            "wq": wq_kt, "wabc": wabc_kt,
            "band01": band2.astype(bf), "suc01": suc2.astype(bf),
        })
    return in_maps


def kernel(x, speaker_ids, W_att, W_pred, W_suc, W_same, W_diff):
    from concourse import bass_utils

    if "nc" not in _cache:
        _cache["nc"] = _build_bass()
    nc = _cache["nc"]

    in_maps = _prep_in_maps({
        "x": x, "speaker_ids": speaker_ids, "W_att": W_att, "W_pred": W_pred,
        "W_suc": W_suc, "W_same": W_same, "W_diff": W_diff})

    res = bass_utils.run_bass_kernel_spmd(nc, in_maps, core_ids=list(range(NCORES)))
    _cache["last_result"] = res
    return np.concatenate([res.results[k]["out"] for k in range(NCORES)], axis=0)


# revision 12
# speedup vs baseline: 1.1750x; 1.1092x over previous
"""DialogueGCN windowed-attention relational GCN on 8 Trainium2 NeuronCores.

Sharding: utterance axis N=16384 split into 8 shards of 2048 rows; each core
gets its shard plus a 128-row halo on each side (zero-padded at the global
edges). The small DxD weights are replicated. No collectives needed.

v2: chunk-centric, transpose-free pipeline. All banded ops are dense matmuls
on a 64-row-shifted chunk grid (17 chunks of 128 halo rows per core), with
attention logits computed DIRECTLY TRANSPOSED into strip space [jj, n]:

  qT        = (x @ W_att)^T            (f32r, from host-transposed x)
  S[r]      = x @ Wr_combined          (3 supports via mask linearity, bf16
                                        storage; S_a carries a ones column
                                        that yields the softmax denominator)
  per chunk c (jj rows), as pairs sharing one PSUM bank:
    T[jj,n] = sum_d xT[d,jj] qT[d,n]   (256 n-cols spanning 2 blocks)
    e       = exp(T - C)               (ScalarE, constant shift: softmax
                                        shift-invariance makes a per-row max
                                        unnecessary in f32/bf16 range)
    c1      = e * band01               (band mask, DVE; c2 = c1 * suc01,
                                        c3 = same-speaker stt on c1)
  per block b: 6 accumulating bf16 matmuls strips^T-contract S -> h | denom,
  then log_softmax finalized per block pair straight out of PSUM with rinv
  folded into activation scale operands.

No PE transposes, no row-max reductions, no f32r rounding copies (f32 DMAs
land directly in f32r tiles).
"""

import os
import numpy as np

N_TOT, D, W, SPK = 16384, 256, 64, 8
NCORES = 8
NC_ROWS = N_TOT // NCORES          # 2048 rows per core
HALO = 128
NH = NC_ROWS + 2 * HALO            # 2304 rows with halo
NBLK = NC_ROWS // 128              # 16 output blocks per core
NCH = NH // 128                    # 18 aligned chunks
NSH = NCH - 1                      # 17 chunks on the 64-shifted grid
NPAIR = (NSH + 1) // 2             # 9 strip pairs (last one is a single)
C_SHIFT = 30.0

_cache = {}


def _build_bass():
    import concourse.tile as tile
    from concourse import bacc, mybir

    f32 = mybir.dt.float32
    f32r = mybir.dt.float32r
    bf16 = mybir.dt.bfloat16
    AX = mybir.AxisListType.X
    OP = mybir.AluOpType
    AF = mybir.ActivationFunctionType

    nc = bacc.Bacc("TRN2", target_bir_lowering=False, debug=False,
                   num_devices=NCORES)

    xt_d = nc.dram_tensor("xt", [2, 128, NH], f32r, kind="ExternalInput").ap()
    spk_d = nc.dram_tensor("spk", [NH], bf16, kind="ExternalInput").ap()
    wq_d = nc.dram_tensor("wq", [128, 2, D], f32r, kind="ExternalInput").ap()
    wabc_d = nc.dram_tensor("wabc", [128, 2, 3, D], f32r, kind="ExternalInput").ap()
    band_d = nc.dram_tensor("band01", [128, 512], bf16, kind="ExternalInput").ap()
    suc_d = nc.dram_tensor("suc01", [128, 512], bf16, kind="ExternalInput").ap()
    out_d = nc.dram_tensor("out", [NC_ROWS, D], f32, kind="ExternalOutput").ap()

    qpad = 128                     # zero columns either side of qT

    with tile.TileContext(nc) as tc:
        from contextlib import ExitStack
        with ExitStack() as ctx:
            const = ctx.enter_context(tc.tile_pool(name="const", bufs=1))
            persist = ctx.enter_context(tc.tile_pool(name="persist", bufs=1))
            work = ctx.enter_context(tc.tile_pool(
                name="work", bufs=int(os.environ.get("KB_WORK", "4"))))
            psum = ctx.enter_context(tc.tile_pool(name="psum", bufs=2, space="PSUM"))

            # Pre-seed the activation table set covering Exp + Ln so the
            # table-load pass never reloads mid-kernel.
            nc.scalar.add_instruction(mybir.InstLoadActFuncSet(
                name=nc.get_next_instruction_name(), ins=[], outs=[],
                act_func_set_id=6))

            # ---- input DMAs, ordered so qT(0) can start ASAP ----
            wq = const.tile([128, 2, D], f32r)
            nc.sync.dma_start(wq, wq_d)
            xT = persist.tile([128, 2, NH], f32r)
            xt_v = xt_d.rearrange("k p n -> p k n")
            # first piece covers qT(0) [128, 640) and S(0..3) [64, 576)
            xsplits = [0, 640, 1216, 1792, NH]
            nc.sync.dma_start(xT[:, :, 0:640], xt_v[:, :, 0:640])
            wabc = const.tile([128, 2, 3, D], f32r)
            nc.sync.dma_start(wabc, wabc_d)
            # speakers (bf16 from host): per-chunk columns and broadcast rows
            spk_col = persist.tile([128, NSH], bf16)
            nc.sync.dma_start(
                spk_col, spk_d[64:64 + NSH * 128].rearrange("(c p) -> p c", p=128))
            spk_row = persist.tile([1, NC_ROWS], bf16)
            nc.sync.dma_start(
                spk_row, spk_d.rearrange("(a b) -> a b", a=1)[:, HALO:HALO + NC_ROWS])
            spk_bc = persist.tile([128, NC_ROWS], bf16)
            nc.gpsimd.partition_broadcast(spk_bc, spk_row)
            for g in range(1, len(xsplits) - 1):
                nc.sync.dma_start(xT[:, :, xsplits[g]:xsplits[g + 1]],
                                  xt_v[:, :, xsplits[g]:xsplits[g + 1]])
            band01 = const.tile([128, 512], bf16)
            nc.sync.dma_start(band01, band_d)
            suc01 = const.tile([128, 512], bf16)
            nc.sync.dma_start(suc01, suc_d)

            # qT padded with zero columns so every strip matmul is 256 wide
            qT = persist.tile([128, 2, qpad + NC_ROWS + qpad], f32r)
            nc.gpsimd.memset(qT[:, :, 0:qpad], 0.0)
            nc.gpsimd.memset(qT[:, :, qpad + NC_ROWS:], 0.0)

            # supports: [p, r, chunk, 264] bf16; S_a ones column at 256
            S = persist.tile([128, 3, NSH, 264], bf16)
            nc.gpsimd.memset(S[:, 0, :, 256:257], 1.0)

            # constant exp shift (softmax shift-invariance, no row max)
            cbias = const.tile([128, 1], f32)
            nc.gpsimd.memset(cbias, -C_SHIFT)

            # log-softmax tail staging
            s2_all = persist.tile([128, NBLK], f32)
            rinv_all = persist.tile([128, NBLK], f32)

            # ---- qT: one 512-column group (psq shares PSUM tag with strips) ----
            def emit_qT(g):
                nsl = slice(HALO + g * 512, HALO + (g + 1) * 512)
                for dh in (0, 1):
                    psq = psum.tile([128, 512], f32, tag="big", name="psq",
                                    bufs=int(os.environ.get("KB_PBIG", "2")))
                    for k in (0, 1):
                        nc.tensor.matmul(
                            psq, wq[:, k, dh * 128:(dh + 1) * 128],
                            xT[:, k, nsl], start=(k == 0), stop=(k == 1))
                    qeng = nc.scalar.copy if (g + dh) % 2 else nc.vector.tensor_copy
                    qeng(qT[:, dh, qpad + g * 512:qpad + (g + 1) * 512], psq)

            # ---- one support chunk on the 64-shifted grid ----
            sc_state = {}

            def emit_S(c):
                csl = slice(64 + c * 128, 64 + (c + 1) * 128)
                pab = psum.tile([128, 512], f32, tag="sab", name="pab",
                                bufs=int(os.environ.get("KB_PSAB", "3")))
                for k in (0, 1):
                    for i in range(2):
                        nc.tensor.matmul(
                            pab[:, i * 256:(i + 1) * 256], xT[:, k, csl],
                            wabc[:, k, i, :], start=(k == 0), stop=(k == 1),
                            skip_group_check=True)
                smode = os.environ.get("KB_SP", "parity")
                if smode == "act" or (smode == "parity" and c % 2 == 0):
                    seng = nc.scalar.copy
                elif smode == "pool" or (smode == "parity2" and c % 2 == 0):
                    seng = nc.gpsimd.tensor_copy
                else:
                    seng = nc.vector.tensor_copy
                seng(S[:, 0:2, c, 0:D], pab.rearrange("p (i d) -> p i d", i=2))

                # S_c: two chunks share one PSUM bank (tag shared with pab)
                half = c % 2
                if half == 0:
                    sc_state["t"] = psum.tile([128, 512], f32, tag="sab", name="psc",
                                              bufs=int(os.environ.get("KB_PSAB", "3")))
                psc = sc_state["t"]
                for k in (0, 1):
                    nc.tensor.matmul(psc[:, half * 256:(half + 1) * 256],
                                     xT[:, k, csl], wabc[:, k, 2, :],
                                     start=(k == 0), stop=(k == 1),
                                     skip_group_check=True)
                if half == 1 or c == NSH - 1:
                    c0 = c - half
                    n = half + 1
                    wmode = os.environ.get("KB_WC", "pool")
                    weng = (nc.gpsimd.tensor_copy if wmode == "pool" else
                            nc.vector.tensor_copy if wmode == "dve" else
                            nc.scalar.copy)
                    weng(S[:, 2, c0:c0 + n, 0:D],
                         psc[:, 0:n * 256].rearrange("p (i d) -> p i d", i=n))

            # ---- strip pair P: chunks 2P, 2P+1 -> c1/c2/c3 [128, 512] ----
            strips = {}

            def emit_strips(P):
                cs = [c for c in (2 * P, 2 * P + 1) if c < NSH]
                n = len(cs)
                pt = psum.tile([128, 512], f32, tag="big", name="pt",
                               bufs=int(os.environ.get("KB_PBIG", "2")))
                for i, c in enumerate(cs):
                    jsl = slice(64 + c * 128, 64 + (c + 1) * 128)
                    qsl = slice(qpad + (c - 1) * 128, qpad + (c + 1) * 128)
                    for k in (0, 1):
                        nc.tensor.matmul(pt[:, i * 256:(i + 1) * 256],
                                         xT[:, k, jsl], qT[:, k, qsl],
                                         start=(k == 0), stop=(k == 1),
                                         skip_group_check=True)
                w = n * 256
                c1 = work.tile([128, 512], bf16, tag="c1", name="c1")
                nc.scalar.activation(c1[:, 0:w], pt[:, 0:w], AF.Exp,
                                     bias=cbias)
                nc.vector.tensor_tensor(c1[:, 0:w], c1[:, 0:w], band01[:, 0:w],
                                        op=OP.mult)
                c2 = work.tile([128, 512], bf16, tag="c2", name="c2")
                c2eng = nc.gpsimd if os.environ.get("KB_C2", "dve") == "pool" else nc.vector
                c2eng.tensor_tensor(c2[:, 0:w], c1[:, 0:w], suc01[:, 0:w],
                                    op=OP.mult)
                c3 = work.tile([128, 512], bf16, tag="c3", name="c3")
                for i, c in enumerate(cs):
                    nlo = (c - 1) * 128
                    s0 = max(0, -nlo)
                    s1 = min(256, NC_ROWS - nlo)
                    nc.vector.scalar_tensor_tensor(
                        c3[:, i * 256 + s0:i * 256 + s1],
                        in0=spk_bc[:, nlo + s0:nlo + s1],
                        scalar=spk_col[:, c:c + 1],
                        in1=c1[:, i * 256 + s0:i * 256 + s1],
                        op0=OP.is_equal, op1=OP.mult)
                strips[P] = (c1, c2, c3)

            # ---- one 128-row output block ----
            psh_hist = {}

            def strip_slice(tile_trip, c, b):
                # columns of chunk c's strip belonging to block b
                i = c % 2
                f0 = (b - (c - 1)) * 128           # 128 if c==b, 0 if c==b+1
                return tuple(t[:, i * 256 + f0:i * 256 + f0 + 128]
                             for t in tile_trip)

            def emit_block(b):
                psh = psum.tile([128, 257], f32, tag="psh", name="psh",
                                bufs=int(os.environ.get("KB_PSH", "3")))
                mms = []
                for c in (b, b + 1):
                    c1s, c2s, c3s = strip_slice(strips[c // 2], c, b)
                    mms.append((c1s, 0, c))
                    mms.append((c3s, 2, c))
                    mms.append((c2s, 1, c))
                for i, (st, r, c) in enumerate(mms):
                    wid = 257 if r == 0 else D
                    nc.tensor.matmul(psh[:, 0:wid], st, S[:, r, c, 0:wid],
                                     start=(i == 0), stop=(i == len(mms) - 1),
                                     skip_group_check=True)
                psh_hist[b] = psh
                rinv = rinv_all[:, b:b + 1]
                nc.vector.reciprocal(rinv, psh[:, 256:257])
                e2 = work.tile([128, D], bf16, tag="e2")
                nc.scalar.activation(e2, psh[:, 0:D], AF.Exp, scale=rinv,
                                     accum_out=s2_all[:, b:b + 1])

                if b % 2 == 1:
                    g = b // 2
                    gs = slice(g * 2, g * 2 + 2)
                    ln2 = work.tile([128, 2], f32, tag="ln2")
                    nc.scalar.activation(ln2, s2_all[:, gs], AF.Ln)
                    bias2 = work.tile([128, 2], f32, tag="bias2")
                    nc.vector.tensor_scalar_mul(bias2, ln2, -1.0)
                    ob2 = work.tile([128, 2, D], f32, tag="ob2")
                    obm = os.environ.get("KB_OB", "dve")
                    for i in range(2):
                        bb = 2 * g + i
                        if obm == "act":
                            nc.scalar.activation(
                                ob2[:, i, :], psh_hist[bb][:, 0:D], AF.Identity,
                                bias=bias2[:, i:i + 1],
                                scale=rinv_all[:, bb:bb + 1])
                        else:
                            eng = nc.gpsimd if obm == "pool" else nc.vector
                            eng.tensor_scalar(
                                ob2[:, i, :], psh_hist[bb][:, 0:D],
                                scalar1=rinv_all[:, bb:bb + 1],
                                scalar2=bias2[:, i:i + 1],
                                op0=OP.mult, op1=OP.add)
                        del psh_hist[bb]
                    outeng = (nc.gpsimd if os.environ.get("KB_ODMA", "pool") == "pool"
                              else nc.sync)
                    outeng.dma_start(
                        out_d.rearrange("(c p) d -> p c d", p=128)[:, gs, :], ob2)

            # ---- driver: pipelined emission ----
            for P in range(NPAIR):
                if P < 4:
                    emit_qT(P)
                emit_S(2 * P)
                if 2 * P + 1 < NSH:
                    emit_S(2 * P + 1)
                emit_strips(P)
                for b in (2 * P - 2, 2 * P - 1):
                    if 0 <= b < NBLK:
                        emit_block(b)

    nc.compile()
    return nc


def _host_constants():
    # strip-space masks for a chunk pair [128, 2, 256]: within a chunk,
    # column f covers n = 128*(c-1)+f; row p covers j = 128*c+p-64 (shard
    # coords). In-band: j-n in [-64, 64) <=> p < f <= p+128; successor
    # (j < n): f - p in (64, 128].
    p = np.arange(128)[:, None]
    f = np.arange(256)[None, :]
    band = ((p < f) & (f <= p + 128)).astype(np.float32)
    suc = ((f - p > 64) & (f - p <= 128)).astype(np.float32)
    band2 = np.concatenate([band, band], axis=1)
    suc2 = np.concatenate([suc, suc], axis=1)
    return band2, suc2


def _prep_in_maps(np_inputs):
    import ml_dtypes
    bf = ml_dtypes.bfloat16
    x = np.asarray(np_inputs["x"], dtype=np.float32)
    spk = np.asarray(np_inputs["speaker_ids"]).astype(np.float32)
    W_att = np.asarray(np_inputs["W_att"], dtype=np.float32)
    W_pred = np.asarray(np_inputs["W_pred"], dtype=np.float32)
    W_suc = np.asarray(np_inputs["W_suc"], dtype=np.float32)
    W_same = np.asarray(np_inputs["W_same"], dtype=np.float32)
    W_diff = np.asarray(np_inputs["W_diff"], dtype=np.float32)

    band2, suc2 = _host_constants()
    wa = W_pred + W_diff
    wb = W_suc - W_pred
    wc = W_same - W_diff
    # k-tile layouts [p, k, ...]
    wq_kt = np.ascontiguousarray(W_att.reshape(2, 128, D).transpose(1, 0, 2))
    wabc = np.stack([wa, wb, wc], axis=1)              # [256, 3, D]
    wabc_kt = np.ascontiguousarray(
        wabc.reshape(2, 128, 3, D).transpose(1, 0, 2, 3))

    xp = np.zeros((N_TOT + 2 * HALO, D), dtype=np.float32)
    xp[HALO:HALO + N_TOT] = x
    spkp = np.full((N_TOT + 2 * HALO,), -1.0, dtype=np.float32)
    spkp[HALO:HALO + N_TOT] = spk

    in_maps = []
    for k in range(NCORES):
        r0 = k * NC_ROWS
        in_maps.append({
            "xt": np.ascontiguousarray(
                xp[r0:r0 + NH].T.reshape(2, 128, NH)),
            "spk": np.ascontiguousarray(spkp[r0:r0 + NH]).astype(bf),
            "wq": wq_kt, "wabc": wabc_kt,
            "band01": band2.astype(bf), "suc01": suc2.astype(bf),
        })
    return in_maps


def kernel(x, speaker_ids, W_att, W_pred, W_suc, W_same, W_diff):
    from concourse import bass_utils

    if "nc" not in _cache:
        _cache["nc"] = _build_bass()
    nc = _cache["nc"]

    in_maps = _prep_in_maps({
        "x": x, "speaker_ids": speaker_ids, "W_att": W_att, "W_pred": W_pred,
        "W_suc": W_suc, "W_same": W_same, "W_diff": W_diff})

    res = bass_utils.run_bass_kernel_spmd(nc, in_maps, core_ids=list(range(NCORES)))
    _cache["last_result"] = res
    return np.concatenate([res.results[k]["out"] for k in range(NCORES)], axis=0)


# revision 14
# speedup vs baseline: 1.3669x; 1.1633x over previous
"""DialogueGCN windowed-attention relational GCN on 8 Trainium2 NeuronCores.

Sharding: utterance axis N=16384 split into 8 shards of 2048 rows; each core
gets its shard plus a 128-row halo on each side (zero-padded at the global
edges). The small DxD weights are replicated. No collectives needed.

v2: chunk-centric, transpose-free pipeline. All banded ops are dense matmuls
on a 64-row-shifted chunk grid (17 chunks of 128 halo rows per core), with
attention logits computed DIRECTLY TRANSPOSED into strip space [jj, n]:

  qT        = (x @ W_att)^T            (f32r, from host-transposed x)
  S[r]      = x @ Wr_combined          (3 supports via mask linearity, bf16
                                        storage; S_a carries a ones column
                                        that yields the softmax denominator)
  per chunk c (jj rows), as pairs sharing one PSUM bank:
    T[jj,n] = sum_d xT[d,jj] qT[d,n]   (256 n-cols spanning 2 blocks)
    e       = exp(T - C)               (ScalarE, constant shift: softmax
                                        shift-invariance makes a per-row max
                                        unnecessary in f32/bf16 range)
    c1      = e * band01               (band mask, DVE; c2 = c1 * suc01,
                                        c3 = same-speaker stt on c1)
  per block b: 6 accumulating bf16 matmuls strips^T-contract S -> h | denom,
  then log_softmax finalized per block pair straight out of PSUM with rinv
  folded into activation scale operands.

No PE transposes, no row-max reductions, no f32r rounding copies (f32 DMAs
land directly in f32r tiles).
"""

import os
import numpy as np

N_TOT, D, W, SPK = 16384, 256, 64, 8
NCORES = 8
NC_ROWS = N_TOT // NCORES          # 2048 rows per core
HALO = 128
NH = NC_ROWS + 2 * HALO            # 2304 rows with halo
NBLK = NC_ROWS // 128              # 16 output blocks per core
NCH = NH // 128                    # 18 aligned chunks
NSH = NCH - 1                      # 17 chunks on the 64-shifted grid
NPAIR = (NSH + 1) // 2             # 9 strip pairs (last one is a single)
C_SHIFT = 30.0

_cache = {}


def _build_bass():
    import concourse.tile as tile
    from concourse import bacc, mybir

    f32 = mybir.dt.float32
    f32r = mybir.dt.float32r
    bf16 = mybir.dt.bfloat16
    AX = mybir.AxisListType.X
    OP = mybir.AluOpType
    AF = mybir.ActivationFunctionType

    nc = bacc.Bacc("TRN2", target_bir_lowering=False, debug=False,
                   num_devices=NCORES)

    xt_d = nc.dram_tensor("xt", [2, 128, NH], f32r, kind="ExternalInput").ap()
    spk_d = nc.dram_tensor("spk", [NH], bf16, kind="ExternalInput").ap()
    wq_d = nc.dram_tensor("wq", [128, 2, D], f32r, kind="ExternalInput").ap()
    wabc_d = nc.dram_tensor("wabc", [128, 2, 3, D], f32r, kind="ExternalInput").ap()
    band_d = nc.dram_tensor("band01", [128, 512], bf16, kind="ExternalInput").ap()
    suc_d = nc.dram_tensor("suc01", [128, 512], bf16, kind="ExternalInput").ap()
    out_d = nc.dram_tensor("out", [NC_ROWS, D], f32, kind="ExternalOutput").ap()

    qpad = 128                     # zero columns either side of qT

    with tile.TileContext(nc) as tc:
        from contextlib import ExitStack
        with ExitStack() as ctx:
            const = ctx.enter_context(tc.tile_pool(name="const", bufs=1))
            persist = ctx.enter_context(tc.tile_pool(name="persist", bufs=1))
            work = ctx.enter_context(tc.tile_pool(
                name="work", bufs=int(os.environ.get("KB_WORK", "4"))))
            psum = ctx.enter_context(tc.tile_pool(name="psum", bufs=2, space="PSUM"))

            # Pre-seed the activation table set covering Exp + Ln so the
            # table-load pass never reloads mid-kernel.
            nc.scalar.add_instruction(mybir.InstLoadActFuncSet(
                name=nc.get_next_instruction_name(), ins=[], outs=[],
                act_func_set_id=6))

            # ---- input DMAs, ordered so qT(0) can start ASAP ----
            wq = const.tile([128, 2, D], f32r)
            nc.sync.dma_start(wq, wq_d)
            xT = persist.tile([128, 2, NH], f32r)
            xt_v = xt_d.rearrange("k p n -> p k n")
            # first piece covers qT(0) [128, 640) and S(0..3) [64, 576)
            xsplits = [0, 640, 1216, 1792, NH]
            nc.sync.dma_start(xT[:, :, 0:640], xt_v[:, :, 0:640])
            wabc = const.tile([128, 2, 3, D], f32r)
            nc.sync.dma_start(wabc, wabc_d)
            # speakers (bf16 from host): per-chunk columns and broadcast rows
            spk_col = persist.tile([128, NSH], bf16)
            nc.sync.dma_start(
                spk_col, spk_d[64:64 + NSH * 128].rearrange("(c p) -> p c", p=128))
            spk_row = persist.tile([1, NC_ROWS], bf16)
            nc.sync.dma_start(
                spk_row, spk_d.rearrange("(a b) -> a b", a=1)[:, HALO:HALO + NC_ROWS])
            spk_bc = persist.tile([128, NC_ROWS], bf16)
            nc.gpsimd.partition_broadcast(spk_bc, spk_row)
            for g in range(1, len(xsplits) - 1):
                nc.sync.dma_start(xT[:, :, xsplits[g]:xsplits[g + 1]],
                                  xt_v[:, :, xsplits[g]:xsplits[g + 1]])
            band01 = const.tile([128, 512], bf16)
            nc.sync.dma_start(band01, band_d)
            suc01 = const.tile([128, 512], bf16)
            nc.sync.dma_start(suc01, suc_d)

            # qT padded with zero columns so every strip matmul is 256 wide
            qT = persist.tile([128, 2, qpad + NC_ROWS + qpad], f32r)
            nc.gpsimd.memset(qT[:, :, 0:qpad], 0.0)
            nc.gpsimd.memset(qT[:, :, qpad + NC_ROWS:], 0.0)

            # supports: [p, r, chunk, 264] bf16; S_a ones column at 256
            S = persist.tile([128, 3, NSH, 264], bf16)
            nc.gpsimd.memset(S[:, 0, :, 256:257], 1.0)

            # constant exp shift (softmax shift-invariance, no row max)
            cbias = const.tile([128, 1], f32)
            nc.gpsimd.memset(cbias, -C_SHIFT)

            # log-softmax tail staging
            s2_all = persist.tile([128, NBLK], f32)
            rinv_all = persist.tile([128, NBLK], f32)

            # per-block softmax denominators, one persistent PSUM bank
            pden = psum.tile([128, NBLK], f32, tag="pden", name="pden", bufs=1)

            # PE warmup: fill the DMA prologue with throwaway matmuls so the
            # p-state ramp is done when real work lands
            nwarm = int(os.environ.get("KB_WARM", "6"))
            if nwarm:
                warm = const.tile([128, 512], bf16)
                nc.gpsimd.memset(warm, 0.0)
                for i in range(nwarm):
                    psw = psum.tile([128, 512], f32, tag="big", name="psw",
                                    bufs=int(os.environ.get("KB_PBIG", "2")))
                    nc.tensor.matmul(psw, warm[:, 0:128], warm,
                                     start=True, stop=True)

            # ---- qT: one 512-column group (psq shares PSUM tag with strips) ----
            def emit_qT(g):
                nsl = slice(HALO + g * 512, HALO + (g + 1) * 512)
                for dh in (0, 1):
                    psq = psum.tile([128, 512], f32, tag="big", name="psq",
                                    bufs=int(os.environ.get("KB_PBIG", "2")))
                    for k in (0, 1):
                        nc.tensor.matmul(
                            psq, wq[:, k, dh * 128:(dh + 1) * 128],
                            xT[:, k, nsl], start=(k == 0), stop=(k == 1))
                    qeng = nc.scalar.copy if (g + dh) % 2 else nc.vector.tensor_copy
                    qeng(qT[:, dh, qpad + g * 512:qpad + (g + 1) * 512], psq)

            # ---- one support chunk on the 64-shifted grid ----
            sc_state = {}

            def emit_S(c):
                csl = slice(64 + c * 128, 64 + (c + 1) * 128)
                pab = psum.tile([128, 512], f32, tag="sab", name="pab",
                                bufs=int(os.environ.get("KB_PSAB", "3")))
                for k in (0, 1):
                    for i in range(2):
                        nc.tensor.matmul(
                            pab[:, i * 256:(i + 1) * 256], xT[:, k, csl],
                            wabc[:, k, i, :], start=(k == 0), stop=(k == 1),
                            skip_group_check=True)
                smode = os.environ.get("KB_SP", "parity")
                if smode == "act" or (smode == "parity" and c % 2 == 0):
                    seng = nc.scalar.copy
                elif smode == "pool" or (smode == "parity2" and c % 2 == 0):
                    seng = nc.gpsimd.tensor_copy
                else:
                    seng = nc.vector.tensor_copy
                seng(S[:, 0:2, c, 0:D], pab.rearrange("p (i d) -> p i d", i=2))

                # S_c: two chunks share one PSUM bank (tag shared with pab)
                half = c % 2
                if half == 0:
                    sc_state["t"] = psum.tile([128, 512], f32, tag="sab", name="psc",
                                              bufs=int(os.environ.get("KB_PSAB", "3")))
                psc = sc_state["t"]
                for k in (0, 1):
                    nc.tensor.matmul(psc[:, half * 256:(half + 1) * 256],
                                     xT[:, k, csl], wabc[:, k, 2, :],
                                     start=(k == 0), stop=(k == 1),
                                     skip_group_check=True)
                if half == 1 or c == NSH - 1:
                    c0 = c - half
                    n = half + 1
                    wmode = os.environ.get("KB_WC", "pool")
                    weng = (nc.gpsimd.tensor_copy if wmode == "pool" else
                            nc.vector.tensor_copy if wmode == "dve" else
                            nc.scalar.copy)
                    weng(S[:, 2, c0:c0 + n, 0:D],
                         psc[:, 0:n * 256].rearrange("p (i d) -> p i d", i=n))

            # ---- strip pair P: chunks 2P, 2P+1 -> c1/c2/c3 [128, 512] ----
            strips = {}

            def emit_strips(P):
                cs = [c for c in (2 * P, 2 * P + 1) if c < NSH]
                n = len(cs)
                pt = psum.tile([128, 512], f32, tag="big", name="pt",
                               bufs=int(os.environ.get("KB_PBIG", "2")))
                for i, c in enumerate(cs):
                    jsl = slice(64 + c * 128, 64 + (c + 1) * 128)
                    qsl = slice(qpad + (c - 1) * 128, qpad + (c + 1) * 128)
                    for k in (0, 1):
                        nc.tensor.matmul(pt[:, i * 256:(i + 1) * 256],
                                         xT[:, k, jsl], qT[:, k, qsl],
                                         start=(k == 0), stop=(k == 1),
                                         skip_group_check=True)
                w = n * 256
                c1 = work.tile([128, 512], bf16, tag="c1", name="c1")
                nc.scalar.activation(c1[:, 0:w], pt[:, 0:w], AF.Exp,
                                     bias=cbias)
                nc.vector.tensor_tensor(c1[:, 0:w], c1[:, 0:w], band01[:, 0:w],
                                        op=OP.mult)
                c2 = work.tile([128, 512], bf16, tag="c2", name="c2")
                c2eng = nc.gpsimd if os.environ.get("KB_C2", "dve") == "pool" else nc.vector
                c2eng.tensor_tensor(c2[:, 0:w], c1[:, 0:w], suc01[:, 0:w],
                                    op=OP.mult)
                c3 = work.tile([128, 512], bf16, tag="c3", name="c3")
                for i, c in enumerate(cs):
                    nlo = (c - 1) * 128
                    s0 = max(0, -nlo)
                    s1 = min(256, NC_ROWS - nlo)
                    nc.vector.scalar_tensor_tensor(
                        c3[:, i * 256 + s0:i * 256 + s1],
                        in0=spk_bc[:, nlo + s0:nlo + s1],
                        scalar=spk_col[:, c:c + 1],
                        in1=c1[:, i * 256 + s0:i * 256 + s1],
                        op0=OP.is_equal, op1=OP.mult)
                strips[P] = (c1, c2, c3)

            # ---- one 128-row output block ----
            psh_hist = {}

            def strip_slice(tile_trip, c, b):
                # columns of chunk c's strip belonging to block b
                i = c % 2
                f0 = (b - (c - 1)) * 128           # 128 if c==b, 0 if c==b+1
                return tuple(t[:, i * 256 + f0:i * 256 + f0 + 128]
                             for t in tile_trip)

            def emit_block(b):
                half = b % 2
                if half == 0:
                    psh_hist["t"] = psum.tile(
                        [128, 512], f32, tag="psh", name="psh",
                        bufs=int(os.environ.get("KB_PSH", "2")))
                psh = psh_hist["t"]
                hsl = slice(half * 256, half * 256 + D)
                mms = []
                for c in (b, b + 1):
                    c1s, c2s, c3s = strip_slice(strips[c // 2], c, b)
                    mms.append((c1s, 0, c))
                    mms.append((c3s, 2, c))
                    mms.append((c2s, 1, c))
                for i, (st, r, c) in enumerate(mms):
                    nc.tensor.matmul(psh[:, hsl], st, S[:, r, c, 0:D],
                                     start=(i == 0), stop=(i == len(mms) - 1),
                                     skip_group_check=True)
                # denominator: ones-column of S_a contracted with c1 strips
                for j, c in enumerate((b, b + 1)):
                    c1s = strip_slice(strips[c // 2], c, b)[0]
                    nc.tensor.matmul(pden[:, b:b + 1], c1s, S[:, 0, c, 256:257],
                                     start=(j == 0), stop=(j == 1),
                                     skip_group_check=True)
                rinv = rinv_all[:, b:b + 1]
                nc.vector.reciprocal(rinv, pden[:, b:b + 1])
                e2 = work.tile([128, D], bf16, tag="e2")
                nc.scalar.activation(e2, psh[:, hsl], AF.Exp, scale=rinv,
                                     accum_out=s2_all[:, b:b + 1])

                if half == 1:
                    g = b // 2
                    gs = slice(g * 2, g * 2 + 2)
                    ln2 = work.tile([128, 2], f32, tag="ln2")
                    nc.scalar.activation(ln2, s2_all[:, gs], AF.Ln)
                    ob2 = work.tile([128, 2, D], f32, tag="ob2")
                    obm = os.environ.get("KB_OB", "dve")
                    for i in range(2):
                        bb = 2 * g + i
                        eng = nc.gpsimd if obm == "pool" else nc.vector
                        eng.tensor_scalar(
                            ob2[:, i, :], psh[:, i * 256:i * 256 + D],
                            scalar1=rinv_all[:, bb:bb + 1],
                            scalar2=ln2[:, i:i + 1],
                            op0=OP.mult, op1=OP.subtract)
                    outeng = (nc.gpsimd if os.environ.get("KB_ODMA", "pool") == "pool"
                              else nc.sync)
                    outeng.dma_start(
                        out_d.rearrange("(c p) d -> p c d", p=128)[:, gs, :], ob2)

            # ---- driver: pipelined emission ----
            for P in range(NPAIR):
                if P < 4:
                    emit_qT(P)
                emit_S(2 * P)
                if 2 * P + 1 < NSH:
                    emit_S(2 * P + 1)
                emit_strips(P)
                for b in (2 * P - 2, 2 * P - 1):
                    if 0 <= b < NBLK:
                        emit_block(b)

    nc.compile()
    return nc


def _host_constants():
    # strip-space masks for a chunk pair [128, 2, 256]: within a chunk,
    # column f covers n = 128*(c-1)+f; row p covers j = 128*c+p-64 (shard
    # coords). In-band: j-n in [-64, 64) <=> p < f <= p+128; successor
    # (j < n): f - p in (64, 128].
    p = np.arange(128)[:, None]
    f = np.arange(256)[None, :]
    band = ((p < f) & (f <= p + 128)).astype(np.float32)
    suc = ((f - p > 64) & (f - p <= 128)).astype(np.float32)
    band2 = np.concatenate([band, band], axis=1)
    suc2 = np.concatenate([suc, suc], axis=1)
    return band2, suc2


def _prep_in_maps(np_inputs):
    import ml_dtypes
    bf = ml_dtypes.bfloat16
    x = np.asarray(np_inputs["x"], dtype=np.float32)
    spk = np.asarray(np_inputs["speaker_ids"]).astype(np.float32)
    W_att = np.asarray(np_inputs["W_att"], dtype=np.float32)
    W_pred = np.asarray(np_inputs["W_pred"], dtype=np.float32)
    W_suc = np.asarray(np_inputs["W_suc"], dtype=np.float32)
    W_same = np.asarray(np_inputs["W_same"], dtype=np.float32)
    W_diff = np.asarray(np_inputs["W_diff"], dtype=np.float32)

    band2, suc2 = _host_constants()
    wa = W_pred + W_diff
    wb = W_suc - W_pred
    wc = W_same - W_diff
    # k-tile layouts [p, k, ...]
    wq_kt = np.ascontiguousarray(W_att.reshape(2, 128, D).transpose(1, 0, 2))
    wabc = np.stack([wa, wb, wc], axis=1)              # [256, 3, D]
    wabc_kt = np.ascontiguousarray(
        wabc.reshape(2, 128, 3, D).transpose(1, 0, 2, 3))

    xp = np.zeros((N_TOT + 2 * HALO, D), dtype=np.float32)
    xp[HALO:HALO + N_TOT] = x
    spkp = np.full((N_TOT + 2 * HALO,), -1.0, dtype=np.float32)
    spkp[HALO:HALO + N_TOT] = spk

    in_maps = []
    for k in range(NCORES):
        r0 = k * NC_ROWS
        in_maps.append({
            "xt": np.ascontiguousarray(
                xp[r0:r0 + NH].T.reshape(2, 128, NH)),
            "spk": np.ascontiguousarray(spkp[r0:r0 + NH]).astype(bf),
            "wq": wq_kt, "wabc": wabc_kt,
            "band01": band2.astype(bf), "suc01": suc2.astype(bf),
        })
    return in_maps


def kernel(x, speaker_ids, W_att, W_pred, W_suc, W_same, W_diff):
    from concourse import bass_utils

    if "nc" not in _cache:
        _cache["nc"] = _build_bass()
    nc = _cache["nc"]

    in_maps = _prep_in_maps({
        "x": x, "speaker_ids": speaker_ids, "W_att": W_att, "W_pred": W_pred,
        "W_suc": W_suc, "W_same": W_same, "W_diff": W_diff})

    res = bass_utils.run_bass_kernel_spmd(nc, in_maps, core_ids=list(range(NCORES)))
    _cache["last_result"] = res
    return np.concatenate([res.results[k]["out"] for k in range(NCORES)], axis=0)
